# revision 8
# baseline (speedup 1.0000x reference)
"""DGCN layer (message passing GNN) on 8 Trainium2 NeuronCores via Bass/Tile.

v5. Steady state is pinned at the SWDGE descriptor-generation floor
(~8.1 ns/idx per Q7 pair, 4 pairs), so this version removes the self-loop
edges (6.25% of all gathers) from the SWDGE path entirely:
  - Each core stages its OWN permutation of the bf16 feature table: the
    core's nodes laid out in (window, slot) order as n_windows blocks of
    exactly 128 rows at the FRONT of the table (offsets shared by the
    SPMD program), remaining nodes after. Every window's self-loop rows
    are then table rows [w*128, w*128+128) -> one HWDGE dma_start per
    window on the lightly-loaded Sync engine instead of 128 gather
    descriptors.
  - Gathered (non-self) edges: T_g tiles/window, two gather chunks
    (8 + T_g-8 tiles), sorted by permuted src id with sliding int16
    bases.
  - num_idxs registers hoisted once (was one MOVE per gather).
  - Everything else as before: bf16 table with outdeg^-0.5 folded in,
    4 SWDGE queues, multi-tile DVE sel + 3 tiles/window on Scalar
    (Relu(coef - coef*Abs(iota - r))), bf16 matmuls (FWL), interleaved
    phase 2.
"""

import math

import numpy as np

P = 128
ALPHA = 0.5
N_CORES = 8
GCH = 8  # max tiles per dma_gather (hw limit: <=1024 idxs/inst)
ACT_T = 3  # sel tiles per window built on the scalar engine
IDX_SPAN = 32768


def _wrap_idx16(flat):
    """dma_gather index layout: entry k -> partition k%16, column k//16,
    replicated across the 8 gpsimd core groups (partitions 16-127)."""
    n = flat.shape[-1]
    assert n % 16 == 0
    cols = n // 16
    w = np.asarray(flat, np.int16).reshape(cols, 16).T  # [16, cols]
    return np.tile(w, (8, 1))  # [128, cols]


def _prep_host(h, src, dst, distance, n_cores):
    N, D = h.shape
    E = src.shape[0]
    npc = N // n_cores
    n_windows = (npc + P - 1) // P

    src = np.asarray(src).astype(np.int64)
    dst = np.asarray(dst).astype(np.int64)
    distance = np.asarray(distance)

    out_deg = np.bincount(src, minlength=N).astype(np.float64)
    in_deg = np.bincount(dst, minlength=N).astype(np.float64)
    coef_all = (np.float64(ALPHA) ** distance.astype(np.float64)).astype(np.float32)
    s_all = in_deg**-1.5

    # Deal nodes (sorted by in-degree) into n_cores*n_windows bins in rounds.
    n_bins = n_cores * n_windows
    order_nodes = np.argsort(-in_deg, kind="stable")
    node_bin = np.empty(N, np.int64)
    node_slot = np.empty(N, np.int64)
    esum = np.zeros(n_bins, np.int64)
    fill = np.zeros(n_bins, np.int64)
    pos = 0
    while pos < N:
        take = min(n_bins, N - pos)
        nodes_r = order_nodes[pos : pos + take]
        bins_r = np.argsort(esum, kind="stable")[:take]
        node_bin[nodes_r] = bins_r
        node_slot[nodes_r] = fill[bins_r]
        fill[bins_r] += 1
        esum[bins_r] += in_deg[nodes_r].astype(np.int64)
        pos += take
    node_core = node_bin // n_windows
    node_window = node_bin % n_windows

    # Per-core table: the core's own nodes as n_windows blocks of exactly
    # P rows (empty slots duplicate node 0) at the front, then the FULL
    # original table. Gathers index row NOWN + src (no per-core index
    # remap); the front blocks serve the per-window self-loop DMAs.
    NOWN = n_windows * P
    own = np.zeros((n_cores, n_windows, P), np.int64)
    has_node = np.zeros((n_cores, n_windows, P), bool)
    own[node_core, node_window, node_slot] = np.arange(N)
    has_node[node_core, node_window, node_slot] = True
    NTAB = NOWN + N

    core_of = node_core[dst]
    w_of = node_window[dst]
    r_of = node_slot[dst].astype(np.float32)

    is_self = np.arange(E) < N  # the guaranteed self-loop block
    gmask = ~is_self

    gw = core_of * n_windows + w_of
    gcounts = np.bincount(gw[gmask], minlength=n_bins)
    maxg = int(gcounts.max())
    T_g = max(1, int(math.ceil(maxg / P)))
    cap = T_g * P
    T = T_g + 1  # + self tile
    n_cols = n_windows * T

    # sort gathered edges by (core, window, src)
    psrc_all = NOWN + src  # table row of each edge's src
    key = gw * (1 << 17) + psrc_all
    eg = np.flatnonzero(gmask)
    order = eg[np.argsort(key[eg], kind="stable")]
    sgw = gw[order]
    win_start = np.concatenate([[0], np.cumsum(gcounts)[:-1]])
    q = np.arange(order.size, dtype=np.int64) - win_start[sgw]

    core_arr = sgw // n_windows
    w_arr = sgw % n_windows

    srcs = np.full((n_cores, n_windows, cap), -1, np.int64)
    rofs_e = np.zeros((n_cores, n_windows, cap), np.float32)
    coef_e = np.zeros((n_cores, n_windows, cap), np.float32)
    srcs[core_arr, w_arr, q] = psrc_all[order]
    rofs_e[core_arr, w_arr, q] = r_of[order]
    coef_e[core_arr, w_arr, q] = coef_all[order]

    # gather chunks over T_g tiles
    chunk_tiles = [min(GCH, T_g)]
    if T_g > GCH:
        chunk_tiles.append(T_g - GCH)
    assert sum(chunk_tiles) == T_g and all(t <= GCH for t in chunk_tiles)
    n_chunks = len(chunk_tiles)
    chunk_p0 = [0] + [t * P for t in chunk_tiles[:-1]]
    for i in range(1, n_chunks):
        chunk_p0[i] += chunk_p0[i - 1]

    bases = np.zeros((n_windows, n_chunks), np.int64)
    for ch in range(n_chunks):
        p0 = chunk_p0[ch]
        assert (gcounts.reshape(n_cores, n_windows) > p0).all()
        bases[:, ch] = srcs[:, :, p0].min(axis=0)
    for ch in range(n_chunks):
        p0 = chunk_p0[ch]
        p1 = p0 + chunk_tiles[ch] * P
        blk = srcs[:, :, p0:p1]
        pad = blk < 0
        blk[pad] = np.broadcast_to(bases[None, :, ch, None], blk.shape)[pad]
        rel = blk - bases[None, :, ch, None]
        assert rel.min() >= 0 and rel.max() < IDX_SPAN, (
            f"chunk {ch} span {rel.max()} exceeds int16 gather reach"
        )

    # tile-major r/coef layouts; tile T_g is the self tile: r = iota,
    # coef = alpha^dist of the node's self-loop edge (edge id == node id)
    rofs = np.zeros((n_cores, P, n_windows, T), np.float32)
    coef = np.zeros((n_cores, P, n_windows, T), np.float32)
    rofs[:, :, :, :T_g] = rofs_e.reshape(
        n_cores, n_windows, T_g, P
    ).transpose(0, 3, 1, 2)
    coef[:, :, :, :T_g] = coef_e.reshape(
        n_cores, n_windows, T_g, P
    ).transpose(0, 3, 1, 2)
    self_coef = np.where(has_node, coef_all[own], 0.0)  # [cores, win, P]
    rofs[:, :, :, T_g] = np.arange(P, dtype=np.float32)[None, :, None]
    coef[:, :, :, T_g] = self_coef.transpose(0, 2, 1)
    rofs = np.ascontiguousarray(rofs).reshape(n_cores, P, n_cols)
    coef = np.ascontiguousarray(coef).reshape(n_cores, P, n_cols)

    # wrapped idx16 per (window, chunk)
    ccols = [t * P // 16 for t in chunk_tiles]
    wcols = sum(ccols)
    idx16 = np.zeros((n_cores, P, n_windows * wcols), np.int16)
    for c in range(n_cores):
        for w in range(n_windows):
            cb = w * wcols
            for ch in range(n_chunks):
                p0 = chunk_p0[ch]
                p1 = p0 + chunk_tiles[ch] * P
                rel = srcs[c, w, p0:p1] - bases[w, ch]
                idx16[c, :, cb : cb + ccols[ch]] = _wrap_idx16(rel)
                cb += ccols[ch]

    snode = np.ones((n_cores, P, n_windows), np.float32)
    snode[node_core, node_slot, node_window] = s_all.astype(np.float32)

    out_core = node_core
    out_row = node_window * P + node_slot

    base_table = (np.asarray(h, np.float64) * (out_deg**-0.5)[:, None]).astype(
        np.float32
    )
    tables = [
        np.concatenate([base_table[own[c].reshape(-1)], base_table])
        for c in range(n_cores)
    ]

    return (
        tables, idx16, rofs, coef, snode, bases, out_core, out_row,
        n_windows, T_g, chunk_tiles, n_cols, NTAB,
    )


def _build_nc(NTAB, D, n_windows, T_g, chunk_tiles, n_cols, bases):
    import concourse.bacc as bacc
    import concourse.tile as tile
    from concourse import mybir

    f32 = mybir.dt.float32
    bf16 = mybir.dt.bfloat16
    i16 = mybir.dt.int16
    T = T_g + 1
    n_chunks = len(chunk_tiles)
    ccols = [t * P // 16 for t in chunk_tiles]
    wcols = sum(ccols)
    idxtot = n_windows * wcols

    # fconst16 layout: rofs | coef | iota | wmat
    f16tot = 2 * n_cols + P + D
    # fconst32 layout: biasf | snode | negr | coef32 | negc
    f32tot = D + n_windows + 3 * n_cols

    nc = bacc.Bacc(
        None, target_bir_lowering=False, debug=False, num_swdge_queues=4
    )
    h_d = nc.declare_dram_parameter("h", [NTAB, D], bf16, isOutput=False)
    idx_d = nc.declare_dram_parameter("idx16", [P, idxtot], i16, isOutput=False)
    fc16_d = nc.declare_dram_parameter("fconst16", [P, f16tot], bf16, isOutput=False)
    fc32_d = nc.declare_dram_parameter("fconst32", [P, f32tot], f32, isOutput=False)
    out_d = nc.declare_dram_parameter("out", [n_windows * P, D], f32, isOutput=True)

    mult = mybir.AluOpType.mult
    AF = mybir.ActivationFunctionType
    DVE_T = T - ACT_T

    with tile.TileContext(nc) as tc:
        with (
            tc.tile_pool(name="singles", bufs=1) as singles,
            tc.tile_pool(name="g", bufs=14) as gpool,
            tc.tile_pool(name="selfp", bufs=6) as selfpool,
            tc.tile_pool(name="selA", bufs=8) as selApool,
            tc.tile_pool(name="selB", bufs=8) as selBpool,
            tc.tile_pool(name="selC", bufs=5 * ACT_T) as selCpool,
            tc.tile_pool(name="tmpC", bufs=4) as tmpCpool,
            tc.tile_pool(name="agg", bufs=4) as aggpool,
            tc.tile_pool(name="psum", bufs=6, space="PSUM") as psumpool,
            tc.tile_pool(name="psum2", bufs=2, space="PSUM") as psum2pool,
            tc.tile_pool(name="outp", bufs=3) as outpool,
        ):
            idx_sb = singles.tile([P, idxtot], i16)
            hd = min(6 * wcols, idxtot)
            nc.sync.dma_start(out=idx_sb[:, :hd], in_=idx_d[:, :hd])
            fc16_sb = singles.tile([P, f16tot], bf16)
            nc.sync.dma_start(out=fc16_sb[:], in_=fc16_d[:])
            if hd < idxtot:
                mid = hd + (idxtot - hd) // 2
                nc.sync.dma_start(out=idx_sb[:, hd:mid], in_=idx_d[:, hd:mid])
                nc.sync.dma_start(out=idx_sb[:, mid:], in_=idx_d[:, mid:])
            fc32_sb = singles.tile([P, f32tot], f32)
            nc.sync.dma_start(out=fc32_sb[:], in_=fc32_d[:])

            r_sb = fc16_sb[:, 0:n_cols]
            c_sb = fc16_sb[:, n_cols : 2 * n_cols]
            o0 = 2 * n_cols
            io_sb = fc16_sb[:, o0 : o0 + P]
            w_sb = fc16_sb[:, o0 + P : o0 + P + D]
            b_sb = fc32_sb[:, 0:D]
            s_sb = fc32_sb[:, D : D + n_windows]
            q0 = D + n_windows
            nr_sb = fc32_sb[:, q0 : q0 + n_cols]
            cf_sb = fc32_sb[:, q0 + n_cols : q0 + 2 * n_cols]
            ncf_sb = fc32_sb[:, q0 + 2 * n_cols : q0 + 3 * n_cols]

            nreg = [nc.gpsimd.to_reg(t * P) for t in chunk_tiles]

            # least-loaded queue assignment: with alternating chunk sizes,
            # round-robin sends all big chunks to queues 0/2 and all small
            # ones to 1/3, so the big queues pace every round. Balance by
            # cumulative idx count instead (deterministic, compile-time).
            qload = [0, 0, 0, 0]
            for w in range(n_windows):
                chunks = []
                cb = w * wcols
                for ch in range(n_chunks):
                    nt = chunk_tiles[ch]
                    g = gpool.tile([P, GCH, P], bf16, tag="g")
                    b = int(bases[w, ch])
                    qn = min(range(4), key=lambda i: (qload[i], i))
                    qload[qn] += nt * P
                    nc.gpsimd.dma_gather(
                        g[:, :nt, :],
                        h_d[b : min(b + IDX_SPAN, NTAB), :],
                        idx_sb[:, cb : cb + ccols[ch]],
                        nt * P,
                        nreg[ch],
                        P,
                        single_packet=False,
                        queue_num=qn,
                    )
                    cb += ccols[ch]
                    chunks.append(g)
                selft = selfpool.tile([P, P], bf16, tag="selfp")
                nc.sync.dma_start(
                    out=selft[:], in_=h_d[w * P : (w + 1) * P, :]
                )

                # sel tiles 0..DVE_T-1 on DVE in <=8-tile groups
                sel_dve = []
                t0g = w * T
                offs = 0
                while offs < DVE_T:
                    ng = min(GCH, DVE_T - offs)
                    if ng > 5:
                        sel = selApool.tile([P, GCH, P], bf16, tag="selA")
                    else:
                        sel = selBpool.tile([P, 5, P], bf16, tag="selB")
                    t0 = t0g + offs
                    rb = r_sb[:, t0 : t0 + ng].unsqueeze(2).broadcast_to([P, ng, P])
                    iob = io_sb.unsqueeze(1).broadcast_to([P, ng, P])
                    nc.vector.tensor_tensor(
                        out=sel[:, :ng, :], in0=rb, in1=iob,
                        op=mybir.AluOpType.is_equal,
                    )
                    cb16 = c_sb[:, t0 : t0 + ng].unsqueeze(2).broadcast_to([P, ng, P])
                    nc.vector.tensor_tensor(
                        out=sel[:, :ng, :], in0=sel[:, :ng, :], in1=cb16, op=mult
                    )
                    sel_dve.append((offs, ng, sel))
                    offs += ng

                # sel tiles DVE_T..T-1 on the scalar engine
                sel_act = []
                for j in range(DVE_T, T):
                    t = t0g + j
                    tmp = tmpCpool.tile([P, P], bf16, tag="tmpC")
                    nc.scalar.activation(
                        out=tmp[:], in_=io_sb, func=AF.Abs,
                        bias=nr_sb[:, t : t + 1], scale=1.0,
                    )
                    selc = selCpool.tile([P, P], bf16, tag="selC")
                    nc.scalar.activation(
                        out=selc[:], in_=tmp[:], func=AF.Relu,
                        bias=cf_sb[:, t : t + 1], scale=ncf_sb[:, t : t + 1],
                    )
                    sel_act.append(selc)

                ps = psumpool.tile([P, P], f32)
                for j in range(T):
                    if j < T_g:
                        if j < chunk_tiles[0]:
                            lhsT = chunks[0][:, j, :]
                        else:
                            lhsT = chunks[1][:, j - chunk_tiles[0], :]
                    else:
                        lhsT = selft[:]
                    if j < DVE_T:
                        for offs, ng, sel in sel_dve:
                            if offs <= j < offs + ng:
                                rhs = sel[:, j - offs, :]
                                break
                    else:
                        rhs = sel_act[j - DVE_T][:]
                    nc.tensor.matmul(
                        out=ps[:], lhsT=lhsT, rhs=rhs,
                        start=(j == 0), stop=(j == T - 1),
                    )

                agg = aggpool.tile([P, P], bf16, tag="agg")
                nc.scalar.copy(out=agg[:], in_=ps[:])
                ps2 = psum2pool.tile([P, D], f32)
                nc.tensor.matmul(
                    out=ps2[:], lhsT=agg[:], rhs=w_sb, start=True, stop=True
                )
                o = outpool.tile([P, D], f32)
                nc.vector.tensor_tensor(
                    out=o[:],
                    in0=ps2[:],
                    in1=s_sb[:, w : w + 1].to_broadcast([P, D]),
                    op=mult,
                )
                nc.vector.tensor_add(out=o[:], in0=o[:], in1=b_sb)
                nc.sync.dma_start(out=out_d[w * P : (w + 1) * P, :], in_=o[:])

    nc.compile()
    return nc


def kernel(h, src, dst, distance, weight, bias, _trace=False):
    import ml_dtypes
    from concourse.bass_utils import run_bass_kernel_spmd

    h = np.ascontiguousarray(np.asarray(h, dtype=np.float32))
    weight = np.asarray(weight, dtype=np.float32)
    bias = np.asarray(bias, dtype=np.float32)
    N, D = h.shape

    (
        tables, idx16, rofs, coef, snode, bases, out_core, out_row,
        n_windows, T_g, chunk_tiles, n_cols, NTAB,
    ) = _prep_host(h, src, dst, distance, N_CORES)

    bf = ml_dtypes.bfloat16
    iota = np.broadcast_to(np.arange(P, dtype=np.float32)[None, :], (P, P))
    biasf = np.broadcast_to(bias[None, :], (P, D)).astype(np.float32)

    nc = _build_nc(NTAB, D, n_windows, T_g, chunk_tiles, n_cols, bases)

    in_maps = []
    for c in range(N_CORES):
        fconst16 = np.concatenate(
            [rofs[c], coef[c], iota, weight], axis=1
        ).astype(bf)
        fconst32 = np.concatenate(
            [biasf, snode[c], -rofs[c], coef[c], -coef[c]], axis=1
        ).astype(np.float32)
        in_maps.append(
            {
                "h": np.ascontiguousarray(tables[c].astype(bf)),
                "idx16": np.ascontiguousarray(idx16[c]),
                "fconst16": np.ascontiguousarray(fconst16),
                "fconst32": np.ascontiguousarray(fconst32),
            }
        )

    res = run_bass_kernel_spmd(nc, in_maps, list(range(N_CORES)), trace=_trace)

    stacked = np.stack([res.results[c]["out"] for c in range(N_CORES)])
    out = stacked[out_core, out_row].astype(np.float32)

    if _trace:
        return out, res
    return out
